# revision 13
# baseline (speedup 1.0000x reference)
"""LogNormal CRPS loss kernel for Trainium2 (8 NeuronCores, data-parallel over N).

Math: crps_n = mean_s|x_s - y| - (1/S^2) * sum_{i<j}(x_(j) - x_(i)),
with x = exp(mu + sigma*z).  The pairwise term uses the sorted-order identity
sum_{i<j}(x_(j)-x_(i)) = sum_k (2k-S+1) x_(k).  Since exp(mu+sigma*z) is
monotone in z (sigma>0), sorting the fp16-cast noise z per column gives the
sample order; exp is applied after the sort.  The sort is a bitonic network
whose comparator patterns are expressed in a rol1 bit-permuted slot space so
27/28 compare-exchange rounds have innermost step=1 APs (DVE 2x_1P on fp16).

Layout per core: batch elements on 128 partitions x 32 groups; 128 sort slots
per group along the free dim (slots 100..127 padded with +BIG).
"""

import numpy as np

import concourse.bass as bass
import concourse.bacc as bacc
import concourse.mybir as mybir
from concourse.tile import TileContext
from concourse.bass_utils import run_bass_kernel_spmd

S = 100
N = 32768
NCORES = 8
NL = N // NCORES          # 4096 batch elements per core
G = NL // 128             # 32 groups
NSLOT = 128
PITCH = G * NSLOT         # free-dim pitch of the big tiles
EPS = 1e-6
BIG16 = 30000.0           # pad key, sorts above any real z
F32 = mybir.dt.float32
F16 = mybir.dt.float16


def _rol1(v):
    return ((v << 1) | (v >> 6)) & 127


def _substage_aps():
    """(lo_dims, lo_off, hi_dims, hi_off) per substage, for ONE 128-slot group.
    Block dims that tile the full 128-slot group are merged with the group dim
    by the caller (multiply count by G)."""
    out = []
    for k in range(1, 8):
        if k == 7:
            out.append(([(2, 64)], 0, [(-2, 64)], 127))
        elif k == 1:
            out.append(([(4, 32), (1, 2)], 0, [(4, 32), (1, 2)], 2))
        else:
            blk = (2 ** (k + 1), 2 ** (6 - k))
            out.append((
                [blk, (2, 2 ** (k - 1)), (1, 2)], 0,
                [blk, (-2, 2 ** (k - 1)), (1, 2)], 2 ** (k + 1) - 2,
            ))
        for j in range(k - 2, -1, -1):
            D = 2 ** (j + 1)
            out.append(([(2 * D, 64 // D), (1, D)], 0,
                        [(2 * D, 64 // D), (1, D)], D))
    return out


def _merge_groups(dims, ng=G):
    """Prepend/merge the group dim (step 128, count ng) into a one-group dim
    list.  The leading block dim tiles [0,128) so it merges exactly."""
    step0, cnt0 = dims[0]
    if step0 * cnt0 == NSLOT:
        return [(step0, cnt0 * ng)] + list(dims[1:])
    return [(NSLOT, ng)] + list(dims)


def weight_vector():
    """w_store[slot]: weight (2r - S + 1) of the rank r stored in that slot
    after the permuted sort; 0 for pad slots."""
    w = np.zeros(NSLOT, dtype=np.float32)
    for r in range(S):
        w[_rol1(r)] = 2 * r - S + 1
    return w


def build_kernel():
    nc = bacc.Bacc("TRN2", target_bir_lowering=False, debug=False)
    noise = nc.dram_tensor("noise", [S, NL], F32, kind="ExternalInput")
    mu = nc.dram_tensor("mu", [NL], F32, kind="ExternalInput")
    sigma = nc.dram_tensor("sigma", [NL], F32, kind="ExternalInput")
    target = nc.dram_tensor("target", [NL], F32, kind="ExternalInput")
    wrep = nc.dram_tensor("wrep", [128, NSLOT], F32, kind="ExternalInput")
    out = nc.dram_tensor("out", [128, 2], F32, kind="ExternalOutput")

    NCHUNK = 2
    GC = G // NCHUNK               # groups per chunk
    CW = GC * NSLOT                # free-dim width per chunk

    with TileContext(nc) as tc:
        with tc.tile_pool(name="main", bufs=1) as pool:
            z32 = pool.tile([128, PITCH], F32)
            z16 = pool.tile([128, PITCH], F16)
            keysA = pool.tile([128, PITCH], F16)
            keysB = pool.tile([128, PITCH], F16)
            srt = pool.tile([128, PITCH], F32)
            scr = pool.tile([128, PITCH], F32)
            scr2 = pool.tile([128, PITCH], F32)
            mus = pool.tile([128, G], F32)
            sgs = pool.tile([128, G], F32)
            ys = pool.tile([128, G], F32)
            yneg = pool.tile([128, G], F32)
            wt = pool.tile([128, NSLOT], F32)
            t1a = pool.tile([128, G], F32)
            t1b = pool.tile([128, G], F32)
            wacc = pool.tile([128, G], F32)
            osb = pool.tile([128, 2], F32)

            def ap(t, off, dims):
                return bass.AP(t[:].tensor, off,
                               [[PITCH, 128]] + [[s, c] for s, c in dims])

            # small loads + clips
            nc.sync.dma_start(mus[:], mu.ap().rearrange("(g p) -> p g", p=128))
            nc.sync.dma_start(sgs[:], sigma.ap().rearrange("(g p) -> p g", p=128))
            nc.sync.dma_start(ys[:], target.ap().rearrange("(g p) -> p g", p=128))
            nc.sync.dma_start(wt[:], wrep.ap())
            nc.vector.tensor_scalar_max(sgs[:], sgs[:], EPS)
            nc.vector.tensor_scalar_max(ys[:], ys[:], EPS)
            nc.vector.tensor_scalar_mul(yneg[:], ys[:], -1.0)
            nc.gpsimd.memset(srt[:], 0.0)

            # prologue per chunk: load, pad, cast, transpose, key transform
            nc.vector.memset(z16[96:128, :], BIG16)
            for c in range(NCHUNK):
                cs = slice(c * CW, (c + 1) * CW)
                nc.sync.dma_start(z32[0:S, cs], noise.ap()[:, cs])
                nc.scalar.copy(z16[0:S, cs], z32[0:S, cs])
                for g in range(c * GC, (c + 1) * GC):
                    nc.sync.dma_start(
                        keysA[:, g * NSLOT:(g + 1) * NSLOT],
                        z16[:, g * NSLOT:(g + 1) * NSLOT],
                        transpose=True,
                    )
                # keys <- sigma*z + mu on real slots (monotone in z, so the
                # sort order is unchanged and the post-sort exp needs no
                # per-group bias/scale).  Pad slots stay at BIG16.  On ACT
                # (Identity with per-partition scale/bias) to spare the DVE.
                for g in range(c * GC, (c + 1) * GC):
                    nc.scalar.activation(
                        keysA[:, g * NSLOT:g * NSLOT + S],
                        keysA[:, g * NSLOT:g * NSLOT + S],
                        mybir.ActivationFunctionType.Identity,
                        bias=mus[:, g:g + 1], scale=sgs[:, g:g + 1])

            # bitonic sort per chunk, ping-pong keysA/keysB (28 substages,
            # even count -> sorted keys end in keysA)
            subs = _substage_aps()
            finals = []
            for c in range(NCHUNK):
                cur, oth = keysA, keysB
                cbase = c * CW
                for lo_d, lo_o, hi_d, hi_o in subs:
                    lod = _merge_groups(lo_d, GC)
                    hid = _merge_groups(hi_d, GC)
                    clo = ap(cur, cbase + lo_o, lod)
                    chi = ap(cur, cbase + hi_o, hid)
                    olo = ap(oth, cbase + lo_o, lod)
                    ohi = ap(oth, cbase + hi_o, hid)
                    nc.vector.tensor_tensor(olo, clo, chi, op=mybir.AluOpType.min)
                    nc.vector.tensor_tensor(ohi, clo, chi, op=mybir.AluOpType.max)
                    cur, oth = oth, cur
                finals.append(cur)

            # post-sort per chunk.  rank r lives at slot rol1(r): ranks 0..63
            # at even slots, 64..99 at odd slots 1..71; pads at odd slots >=73.
            ev = [(NSLOT, GC), (2, 64)]
            od = [(NSLOT, GC), (2, 36)]
            for c in range(NCHUNK):
                cur = finals[c]
                cbase = c * CW
                # sorted samples: one exp per slot-parity over all chunk groups
                for dims, off in ((ev, 0), (od, 1)):
                    nc.scalar.activation(
                        ap(srt, cbase + off, dims), ap(cur, cbase + off, dims),
                        mybir.ActivationFunctionType.Exp)
                # term1 |x - y|: per-group ACT Abs with bias=-y, accum=sum
                for g in range(c * GC, (c + 1) * GC):
                    base = g * NSLOT
                    for dims, off, acc in (([(2, 64)], 0, t1a), ([(2, 36)], 1, t1b)):
                        nc.scalar.activation(
                            ap(scr2, base + off, dims), ap(srt, base + off, dims),
                            mybir.ActivationFunctionType.Abs,
                            bias=yneg[:, g:g + 1], scale=1.0,
                            accum_out=acc[:, g:g + 1])
                # term2 weighted sum: one stt over the whole chunk, with the
                # weight row broadcast across groups via a step-0 AP dim.
                wt_b = bass.AP(wt[:].tensor, 0, [[NSLOT, 128], [0, GC], [1, NSLOT]])
                nc.vector.scalar_tensor_tensor(
                    ap(scr, cbase, [(NSLOT, GC), (1, NSLOT)]),
                    ap(srt, cbase, [(NSLOT, GC), (1, NSLOT)]),
                    1.0,
                    wt_b,
                    op0=mybir.AluOpType.bypass,
                    op1=mybir.AluOpType.mult,
                    accum_out=wacc[:, c:c + 1])

            # per-partition partials: osb[:,0] = sum_g t1, osb[:,1] = sum_g wsum
            nc.vector.tensor_add(t1a[:], t1a[:], t1b[:])
            nc.vector.reduce_sum(osb[:, 0:1], t1a[:], axis=mybir.AxisListType.X)
            nc.vector.reduce_sum(osb[:, 1:2], wacc[:, 0:NCHUNK],
                                 axis=mybir.AxisListType.X)
            nc.sync.dma_start(out.ap(), osb[:])

    nc.compile()
    return nc


_NC_CACHE = {}
_LAST_RESULT = {}


def kernel(mu, sigma, target, noise):
    if "nc" not in _NC_CACHE:
        _NC_CACHE["nc"] = build_kernel()
    nc = _NC_CACHE["nc"]

    wrep = np.tile(weight_vector(), (128, 1)).astype(np.float32)
    in_maps = []
    for c in range(NCORES):
        sl = slice(c * NL, (c + 1) * NL)
        in_maps.append({
            "noise": np.ascontiguousarray(noise[:, sl], dtype=np.float32),
            "mu": np.ascontiguousarray(mu[sl], dtype=np.float32),
            "sigma": np.ascontiguousarray(sigma[sl], dtype=np.float32),
            "target": np.ascontiguousarray(target[sl], dtype=np.float32),
            "wrep": wrep,
        })
    res = run_bass_kernel_spmd(nc, in_maps, core_ids=list(range(NCORES)))
    _LAST_RESULT["exec_time_ns"] = res.exec_time_ns
    _LAST_RESULT["trace"] = (res.instructions_and_trace or (None, None))[1]
    tot = 0.0
    for r in res.results:
        p = r["out"].astype(np.float64)
        tot += (p[:, 0] / S - p[:, 1] / (S * S)).sum()
    return np.float32(tot / N)
